# revision 2
# baseline (speedup 1.0000x reference)
# SAGAN self-attention (B=4, H=W=64, C=64, D=8) on 8 TRN2 NeuronCores — v2.
#
# Sharding: core i = (batch b=i//2, half h=i%2); each core computes 2048
# query rows of the 4096x4096 attention for its batch, fused in SBUF.
#
# v2 vs v1 (93 us): the baseline was PE+ACT serial-bound (~72 us PE busy,
# ~68 us ACT busy, half-overlapped). v2 attacks both:
#  - Scores sT[m,n] = (x P x^T)^T computed with contraction K=64 exactly
#    (no bias row) as TWO CONCURRENT row-tiled matmuls (tile_position (0,0)
#    and (64,0)), 2 key-chunks per 512-cycle span: 2.0x PE on scores.
#    Channels are duplicated to partitions 64-127 (xtd/gpd) to feed the
#    second row-tile.
#  - The score bias exp(d_m) (d = f@bg + bg@bf) is folded into the PV
#    stationary on the HOST (hv' = e^d * hv, ones' = e^d), so exp needs no
#    per-chunk per-partition bias and engine ops can span chunks.
#  - PV runs as FOUR CONCURRENT col-tiled matmuls (M=32, tile_position
#    (0,32j)), 4 key-chunks per span: 4x PE on PV. Partial sums land in
#    psv partition groups 32j; the epilogue stationary (Wv replicated per
#    group + multi-hot denominator collector) absorbs the 4-way reduction
#    for free.
#  - exp is split across BOTH psum-capable engines: ACT native Exp (18 of
#    32 chunks/ntile) and DVE Schraudolph (14 chunks): bits = rint(s *
#    128*log2(e) + 16249) written as int16 = the bf16 encoding of e^s
#    (+-3% interp wiggle, mean-zero; verified rint semantics on HW).
#    GPSIMD has no PSUM port (verified) so 2 engines is the ceiling.
#  - PSUM ring: 6 banks of score chunks (slot = chunk%6), ACT owns slots
#    0-2, DVE 3-5 (period 4 gives ACT 0-3 / DVE 4-5; tail 30-31 to ACT),
#    psv 1 bank, pse 1 bank = 8.
import numpy as np
import ml_dtypes

import concourse.bacc as bacc
import concourse.tile as tile
import concourse.mybir as mybir
from concourse.alu_op_type import AluOpType
from concourse.bass_utils import run_bass_kernel_spmd

F32 = mybir.dt.float32
BF16 = mybir.dt.bfloat16
I16 = mybir.dt.int16
AFT = mybir.ActivationFunctionType

B, HH, WW, C = 4, 64, 64, 64
D = 8
N = HH * WW           # 4096 keys
RPC = N // 2          # 2048 queries per core
NCORES = 8
NT, TN, MC = 4, 512, 32   # 4 n-tiles of 512 queries; 32 key chunks of 128

SCH_A = 184.66496414  # 128 * log2(e)
SCH_B = 16249.0       # 127*128 - 7 (centers the mantissa-interp hump)

# Swath s covers chunks (2s, 2s+1). exp engine: ACT on even swaths plus
# the last one (9 ops), DVE on odd (7) — DVE also carries recip+stt.


def _build():
    nc = bacc.Bacc("TRN2", target_bir_lowering=False, debug=False,
                   num_devices=NCORES)

    xtd = nc.dram_tensor("xtd", [128, N], BF16, kind="ExternalInput").ap()
    gpd = nc.dram_tensor("gpd", [128, RPC], BF16, kind="ExternalInput").ap()
    hvq = nc.dram_tensor("hvq", [128, MC * 32], BF16,
                         kind="ExternalInput").ap()
    xrp = nc.dram_tensor("xrp", [128, RPC // 128 * C], BF16,
                         kind="ExternalInput").ap()
    wv4 = nc.dram_tensor("wv4", [128, C], BF16, kind="ExternalInput").ap()
    e4d = nc.dram_tensor("e4d", [128, 1], BF16, kind="ExternalInput").ap()
    out = nc.dram_tensor("out", [RPC, C], F32, kind="ExternalOutput").ap()

    with tile.TileContext(nc) as tc:
        with tc.tile_pool(name="const", bufs=1) as const:
            XTD = const.tile([128, N], BF16)
            GPD = const.tile([128, RPC], BF16)
            HVQ = const.tile([128, MC * 32], BF16)
            XRP = const.tile([128, RPC // 128 * C], BF16)
            WV4 = const.tile([128, C], BF16)
            E4 = const.tile([128, 1], BF16)

            WUP = const.tile([128, 256], BF16)
            PRE = const.tile([1, 1], F32)

            # input DMAs in first-use order (first blocks are the critical
            # path: swath 0 needs XTD cols 0:256 and GPD cols 0:512)
            nc.sync.dma_start(XTD[:, 0:256], xtd[:, 0:256])
            nc.sync.dma_start(GPD[:, 0:512], gpd[:, 0:512])
            nc.sync.dma_start(XTD[:, 256:1024], xtd[:, 256:1024])
            nc.sync.dma_start(HVQ[:, 0:512], hvq[:, 0:512])
            nc.sync.dma_start(XTD[:, 1024:2048], xtd[:, 1024:2048])
            nc.sync.dma_start(HVQ[:, 512:1024], hvq[:, 512:1024])
            nc.sync.dma_start(XTD[:, 2048:3072], xtd[:, 2048:3072])
            nc.sync.dma_start(XTD[:, 3072:4096], xtd[:, 3072:4096])
            nc.sync.dma_start(WV4[:], wv4[:])
            nc.sync.dma_start(E4[:], e4d[:])
            nc.sync.dma_start(GPD[:, 512:1024], gpd[:, 512:1024])
            nc.sync.dma_start(XRP[:, 0:512], xrp[:, 0:512])
            nc.sync.dma_start(GPD[:, 1024:1536], gpd[:, 1024:1536])
            nc.sync.dma_start(XRP[:, 512:1024], xrp[:, 512:1024])
            nc.sync.dma_start(GPD[:, 1536:2048], gpd[:, 1536:2048])
            nc.vector.memset(WUP[:], 0.0)
            # hoist the ACT exp-table load into the initial DMA wait
            nc.scalar.activation(PRE[:], WUP[0:1, 0:1], AFT.Exp)

            with tc.tile_pool(name="ps", bufs=3, space="PSUM") as psp, \
                 tc.tile_pool(name="psv", bufs=1, space="PSUM") as psvp, \
                 tc.tile_pool(name="pse", bufs=1, space="PSUM") as psep, \
                 tc.tile_pool(name="exp", bufs=16) as exp_pool, \
                 tc.tile_pool(name="vd2p", bufs=2) as vd2p, \
                 tc.tile_pool(name="scolp", bufs=2) as scolp, \
                 tc.tile_pool(name="osbp", bufs=2) as osbp:
                PSV = psvp.tile([128, TN], F32)
                mm = nc.tensor.matmul

                # PE warm-up during the initial DMA wait (HAM ramp)
                wps = psep.tile([128, 260], F32, tag="pse")
                for _ in range(10):
                    mm(wps[:, 0:256], lhsT=WUP[:, 0:128], rhs=WUP[:],
                       start=True, stop=True, skip_group_check=True)

                def epilogue(e, vd2, split_dma=False):
                    pse = psep.tile([128, 260], F32, tag="pse")
                    scol = scolp.tile([128, 4], F32)
                    osb = osbp.tile([128, 4 * C], F32)
                    for nb in range(4):
                        mm(pse[:, 256 + nb:257 + nb],
                           lhsT=vd2[:, nb * 128:(nb + 1) * 128], rhs=E4[:],
                           start=True, stop=True, skip_group_check=True)
                    for nb in range(4):
                        mm(pse[:, nb * 64:(nb + 1) * 64],
                           lhsT=vd2[:, nb * 128:(nb + 1) * 128], rhs=WV4[:],
                           start=True, stop=True, skip_group_check=True)
                    nc.vector.reciprocal(scol[:], pse[:, 256:260])
                    for nb in range(4):
                        nc.vector.scalar_tensor_tensor(
                            osb[:, nb * C:(nb + 1) * C],
                            pse[:, nb * 64:(nb + 1) * 64],
                            scol[:, nb:nb + 1],
                            XRP[:, (e * 4 + nb) * C:(e * 4 + nb + 1) * C],
                            op0=AluOpType.mult, op1=AluOpType.add)
                        if split_dma:
                            r0 = e * 512 + nb * 128
                            nc.sync.dma_start(out[r0:r0 + 128, :],
                                              osb[:, nb * C:(nb + 1) * C])
                    if not split_dma:
                        dst = out[e * 512:(e + 1) * 512, :].rearrange(
                            "(t p) c -> p t c", p=128)
                        nc.sync.dma_start(dst, osb[:].rearrange(
                            "p (t c) -> p t c", c=C))

                def pv_round(q, exs):
                    # PV for the PREVIOUS n-tile's ex tiles: its exp is long
                    # done, so these never stall the PE
                    for j in range(4):
                        ch = 4 * q + j
                        ex = exs[2 * q + j // 2]
                        c0 = (j % 2) * 512
                        mm(PSV[32 * j:32 * j + 32, :],
                           lhsT=HVQ[:, ch * 32:(ch + 1) * 32],
                           rhs=ex[:, c0:c0 + 512],
                           start=(q == 0), stop=(q == 7),
                           tile_position=(0, 32 * j),
                           skip_group_check=True)

                def cast_psv(vd2):
                    # psv -> bf16 on ACT (frees psv for the next n-tile)
                    nc.scalar.activation(vd2[:], PSV[:], AFT.Copy)

                pend_vd2 = None
                prev_exs = None
                for nt in range(NT):
                    q0 = nt * TN
                    prev = nt - 1
                    exs = []
                    for s in range(16):
                        ps = psp.tile([128, 1024], F32)
                        for half, off, ch in ((0, 0, 2 * s),
                                              (64, 512, 2 * s + 1)):
                            mm(ps[:, off:off + 512],
                               lhsT=XTD[half:half + 64,
                                        ch * 128:(ch + 1) * 128],
                               rhs=GPD[half:half + 64, q0:q0 + TN],
                               start=True, stop=True,
                               tile_position=(half, 0),
                               skip_group_check=True)
                        if prev >= 0 and s < 8:
                            pv_round(s, prev_exs)
                        if prev >= 0 and s == 8:
                            pend_vd2 = vd2p.tile([128, TN], BF16)
                            cast_psv(pend_vd2)
                        if prev >= 0 and s == 10:
                            epilogue(prev, pend_vd2)
                        ex = exp_pool.tile([128, 1024], BF16)
                        exs.append(ex)
                        if nt == NT - 1 and s >= 14:
                            # tail latency: split the last exps across BOTH
                            # engines so the drain PVs start sooner
                            nc.scalar.activation(ex[:, 0:512], ps[:, 0:512],
                                                 AFT.Exp)
                            nc.vector.tensor_scalar(
                                ex[:, 512:1024].bitcast(I16), ps[:, 512:1024],
                                SCH_A, SCH_B, AluOpType.mult, AluOpType.add)
                        elif s % 2 == 0 or s == 15:
                            nc.scalar.activation(ex[:], ps[:], AFT.Exp)
                        else:
                            nc.vector.tensor_scalar(
                                ex[:].bitcast(I16), ps[:], SCH_A, SCH_B,
                                AluOpType.mult, AluOpType.add)
                    prev_exs = exs
                # drain: last n-tile's PV + epilogue
                for q in range(8):
                    pv_round(q, prev_exs)
                vd2 = vd2p.tile([128, TN], BF16)
                cast_psv(vd2)
                epilogue(NT - 1, vd2, split_dma=True)
    nc.compile()
    return nc


_CACHE = {}


def _get_compiled():
    if "nc" not in _CACHE:
        _CACHE["nc"] = _build()
    return _CACHE["nc"]


def _make_in_maps(x, Wf, bf, Wg, bg, Wh, bh, Wv, bv, gamma):
    x = np.asarray(x, np.float32)
    Wf = np.asarray(Wf, np.float32)
    Wg = np.asarray(Wg, np.float32)
    Wh = np.asarray(Wh, np.float32)
    Wv = np.asarray(Wv, np.float32)
    bf_ = np.asarray(bf, np.float32)
    bg_ = np.asarray(bg, np.float32)
    bh_ = np.asarray(bh, np.float32)
    bv_ = np.asarray(bv, np.float32)
    g0 = float(np.asarray(gamma, np.float32).reshape(-1)[0])

    xf = x.reshape(B, N, C)
    P = Wf @ Wg.T
    wfbg = Wf @ bg_
    bgbf = float(bg_ @ bf_)
    res_bias = g0 * (bh_ @ Wv + bv_)

    wv4 = np.zeros((128, C), np.float32)
    e4 = np.zeros((128, 1), np.float32)
    for j in range(4):
        wv4[32 * j:32 * j + D] = g0 * Wv
        e4[32 * j + D] = 1.0
    wv4 = wv4.astype(ml_dtypes.bfloat16)
    e4 = e4.astype(ml_dtypes.bfloat16)

    in_maps = []
    for i in range(NCORES):
        b, h = divmod(i, 2)
        r0 = h * RPC
        xq = xf[b]                                  # [4096, 64]
        xtd = np.concatenate([xq.T, xq.T], axis=0)  # [128, 4096]
        gp = P @ xq[r0:r0 + RPC].T                  # [64, 2048]
        gpd = np.concatenate([gp, gp], axis=0)
        d = xq @ wfbg + bgbf                        # [4096] score bias
        ed = np.exp(d)
        hv = (xq @ Wh) * ed[:, None]                # [4096, 8] e^d folded
        hq = np.zeros((MC, 128, 32), np.float32)
        hq[:, :, 0:D] = hv.reshape(MC, 128, D)
        hq[:, :, D] = ed.reshape(MC, 128)
        hq = np.ascontiguousarray(
            hq.transpose(1, 0, 2).reshape(128, MC * 32))
        xr = xq[r0:r0 + RPC] + res_bias
        xrp = np.ascontiguousarray(
            xr.reshape(RPC // 128, 128, C).transpose(1, 0, 2).reshape(
                128, -1))
        in_maps.append({"xtd": xtd.astype(ml_dtypes.bfloat16),
                        "gpd": gpd.astype(ml_dtypes.bfloat16),
                        "hvq": hq.astype(ml_dtypes.bfloat16),
                        "xrp": xrp.astype(ml_dtypes.bfloat16),
                        "wv4": wv4, "e4d": e4})
    return in_maps


def _assemble(results):
    outf = np.empty((B, N, C), np.float32)
    for i in range(NCORES):
        b, h = divmod(i, 2)
        outf[b, h * RPC:(h + 1) * RPC] = results[i]["out"]
    return outf.reshape(B, HH, WW, C)


def run(inputs, **spmd_kwargs):
    nc = _get_compiled()
    in_maps = _make_in_maps(**inputs)
    res = run_bass_kernel_spmd(nc, in_maps, core_ids=list(range(NCORES)),
                               **spmd_kwargs)
    return _assemble(res.results), res


def kernel(**inputs):
    out, _ = run(inputs)
    return out
